# revision 29
# baseline (speedup 1.0000x reference)
"""Multi-head attention with fraction-based RoPE ("stoich RoPE") on 8
Trainium2 NeuronCores.

Sharding: each core owns one (batch, query-half) pair — B=4 batches x 2
query halves = 8 shards.  Every core projects Q for its 1024 query rows
and K/V for the full 2048 keys of its batch (K/V projection is computed
on both cores sharing a batch; the 2x redundancy buys a kernel with no
collectives: the attention output rows owned by a core carry the full
head dimension, so the output projection and bias are entirely local).

Key layout tricks (v2):
  - x is DMA'd once into a persistent SBUF tile (4 slabs of
    [128, 8, 512]); the host permutes the token-chunk order per core so
    the Q projection reuses slabs 0-1 (its own query half) — softmax is
    key-order invariant, so K/V key permutation is harmless.
  - weights are host-pre-tiled to [128, pair, f, 128] so each per-pair
    DMA is 128 x 2KB contiguous lines; pair-0's weights issue ahead of
    the bulk prefetch (x slabs first, Wo deferred) so the first
    projection matmul starts ~6us in.
  - RoPE is emitted per-512-column chunk (spreading DVE work across the
    attention window) as 4 partition-offset multiplies: both inputs read
    at the same base partition, only the output is offset (walrus
    requires equal SBUF input bases).
  - V^T tiles are produced by the DMA XBAR transpose engine
    (dma_start_transpose), freeing the PE array and the ACT/DVE copy
    bandwidth entirely.
  - attention inner loop interleaves the pair's two heads with separate
    scores PSUM tiles so each head's exp gates only its own next scores.
  - PV accumulates 65 meaningful PSUM rows (64 dims + ones row for the
    softmax denominator, zero-padded stationary to M=128 which measures
    faster than M=65); eviction writes unnormalized bf16 attn.
    Denominators sit at partitions 0/32/64/96 of a [128, 512] tile so
    one reciprocal costs ~3.3us (free-size-bound), and the reciprocal is
    broadcast via tiny K=1 bf16 matmuls at matching tile positions —
    NOTE: rescheduling these 32-row-mode matmuls relative to the
    surrounding 128-mode matmuls has repeatedly produced nondeterministic
    corruption; keep them exactly at pair end, before leftover pumping.

Per-core phases: A) QKV projection + RoPE + V transpose per head pair
(pumped into the previous pair's attention), B) attention per pair,
C) output projection.
"""

import contextlib
import ctypes
import sys
import types

import numpy as np
import ml_dtypes

import concourse.bass as bass
import concourse.mybir as mybir
import concourse.tile as tile
from concourse.vector_clock import ScopedClock

# ---------------- problem constants (hardcoded per contract) ----------------
B, T, D = 4, 2048, 1024
H, HD = 16, 64  # heads, head dim
HALF = HD // 2
N_CORES = 8
TQ = T // 2  # query rows per core
P = 128
NQ = 512  # moving-dim tile for matmuls
NPAIR = D // P  # 8 head pairs per core
NB = T // NQ  # 4 x slabs
SCALE = 1.0 / np.sqrt(HD)  # folded into exp()
ROPE_SCALE = 1000.0
ROPE_BASE = 10000.0

F32 = mybir.dt.float32
DT_MM = mybir.dt.bfloat16  # dtype of matmul operands (bfloat16 | float32)

_SO_PATH = "/opt/axon/libaxon_pjrt.so"


# ---------------- axon/NTFF environment shims ----------------
def _ntff_profile_hook():
    try:
        lib = ctypes.CDLL(_SO_PATH)
    except OSError:
        return None
    if not hasattr(lib, "axon_start_nrt_profile"):
        return None
    lib.axon_start_nrt_profile.argtypes = [
        ctypes.POINTER(ctypes.c_int64),
        ctypes.c_size_t,
    ]
    lib.axon_start_nrt_profile.restype = ctypes.c_int64
    lib.axon_stop_nrt_profile.argtypes = [ctypes.c_char_p]
    lib.axon_stop_nrt_profile.restype = ctypes.c_int64

    @contextlib.contextmanager
    def _hook(output_dir, device_ids):
        import jax

        jax.devices()
        if device_ids:
            ids = (ctypes.c_int64 * len(device_ids))(*device_ids)
            rc = lib.axon_start_nrt_profile(ids, len(device_ids))
        else:
            rc = lib.axon_start_nrt_profile(None, 0)
        if rc != 0:
            raise RuntimeError(f"axon_start_nrt_profile rc={rc}")
        try:
            yield
        finally:
            n = lib.axon_stop_nrt_profile(str(output_dir).encode())
            if n < 0:
                raise RuntimeError(f"axon_stop_nrt_profile rc={n}")

    return _hook


def install_shims():
    if "antenv.axon_hooks" not in sys.modules:
        mod = types.ModuleType("antenv.axon_hooks")
        hook = _ntff_profile_hook()
        mod.get_axon_ntff_profile_hook = lambda: hook
        mod.set_axon_ntff_profile_hook = lambda h: None
        sys.modules["antenv.axon_hooks"] = mod
    import concourse.bass_utils as bass_utils

    bass_utils.upload_artifacts = lambda tmpdir: str(tmpdir)

    import os

    if os.environ.get("BASS_LDW_OPT") == "1" and not getattr(
        bass_utils, "_ldw_opt_patched", False
    ):
        orig_run = bass_utils.run_command

        def _run_ldw(argv, **kw):
            argv = [
                "--enable-ldw-opt=true" if a == "--enable-ldw-opt=false" else a
                for a in argv
            ]
            return orig_run(argv, **kw)

        bass_utils.run_command = _run_ldw
        bass_utils._ldw_opt_patched = True


class TileContextSplitDrain(tile.TileContext):
    """This walrus build encodes at most 2 sync waits per CTRL
    instruction; Tile's kernel-tail drain wants one wait per logical
    processor.  Split the waits across single-wait NOPs instead."""

    MAX_WAITS = 1

    def _drain_and_barrier(self, tick_clock, wait_clock):
        nc = self.nc
        carrier = nc.sync.nop(nofuse=True)
        wait_clock.add_sem_waits(
            carrier.ins, ScopedClock({None: tick_clock.global_clock})
        )
        waits = list(carrier.ins.sync_info.on_wait or [])
        if len(waits) > self.MAX_WAITS:
            carrier.ins.sync_info.on_wait[:] = waits[: self.MAX_WAITS]
            for i in range(self.MAX_WAITS, len(waits), self.MAX_WAITS):
                extra = nc.sync.nop(nofuse=True)
                extra.ins.sync_info = mybir.SyncInfo(
                    on_wait=list(waits[i : i + self.MAX_WAITS]), on_update=[]
                )
        nc.sync.drain()
        nc.all_engine_barrier()
        assert self.sems is not None
        popped = nc._tile_sem_poison_stack.pop()
        assert popped is self._sem_poison
        nc.clear_and_free_semaphores(list(self.sems.allocated().values()))
        nc.all_engine_barrier()


def _split_sync_waits(nc, max_waits=1):
    """This walrus build rejects instructions carrying more than ~2 sync
    waits.  Move excess waits onto same-engine NOPs inserted just before
    the instruction (AND semantics are preserved: the engine blocks on
    each carrier in program order)."""
    for f in nc.m.functions:
        for bb in f.blocks:
            out = []
            for inst in bb.instructions:
                si = inst.sync_info
                waits = list(si.on_wait) if si and si.on_wait else []
                if len(waits) > max_waits:
                    for i in range(0, len(waits) - max_waits, max_waits):
                        nop = mybir.InstNoOp(
                            name=nc.get_next_instruction_name(), ins=[], outs=[]
                        )
                        nop.engine = inst.engine
                        nop.sync_info = mybir.SyncInfo(
                            on_wait=list(waits[i : i + max_waits]), on_update=[]
                        )
                        nc.register_instruction(nop, overwrite=True)
                        out.append(nop)
                    si.on_wait[:] = waits[len(waits) - max_waits :]
                out.append(inst)
            bb.instructions[:] = out


# ---------------- device program ----------------
def build_nc(dt_mm=DT_MM):
    nc = bass.Bass(
        "TRN2", target_bir_lowering=False, debug=False, num_devices=N_CORES
    )

    # x slabs: [p, nb, f, t] with per-core chunk permutation (slabs 0-1 =
    # own query half)
    xd = nc.dram_tensor("xd", [P, NB, NPAIR, NQ], dt_mm, kind="ExternalInput")
    # weights pre-tiled [p, pair, f, d]
    wqt = nc.dram_tensor("wqt", [P, NPAIR, NPAIR, P], dt_mm, kind="ExternalInput")
    wkt = nc.dram_tensor("wkt", [P, NPAIR, NPAIR, P], dt_mm, kind="ExternalInput")
    wvt = nc.dram_tensor("wvt", [P, NPAIR, NPAIR, P], dt_mm, kind="ExternalInput")
    wot = nc.dram_tensor("wot", [D, D], dt_mm, kind="ExternalInput")
    bq = nc.dram_tensor("bq", [P, NPAIR], F32, kind="ExternalInput")
    bk = nc.dram_tensor("bk", [P, NPAIR], F32, kind="ExternalInput")
    bv = nc.dram_tensor("bv", [P, NPAIR], F32, kind="ExternalInput")
    bob = nc.dram_tensor("bob", [P, D], F32, kind="ExternalInput")
    csaq = nc.dram_tensor("csaq", [P, TQ], dt_mm, kind="ExternalInput")
    csbq = nc.dram_tensor("csbq", [P, TQ], dt_mm, kind="ExternalInput")  # swapped
    csak = nc.dram_tensor("csak", [P, T], dt_mm, kind="ExternalInput")  # permuted
    csbk = nc.dram_tensor("csbk", [P, T], dt_mm, kind="ExternalInput")  # swapped+perm
    out = nc.dram_tensor("out", [TQ, D], F32, kind="ExternalOutput")

    with TileContextSplitDrain(nc) as tc:
        persist_cm = tc.tile_pool(name="persist", bufs=1)
        persist = persist_cm.__enter__()

        def ptile(shape, dt, tag):
            return persist.tile(shape, dt, tag=tag, name=tag)

        with contextlib.ExitStack() as ctx:
            # ---- persistent tiles ----
            xall = ptile([P, NB, NPAIR, NQ], dt_mm, "xall")
            csaq_t = ptile([P, TQ], dt_mm, "csaq_t")
            csbq_t = ptile([P, TQ], dt_mm, "csbq_t")
            csak_t = ptile([P, T], dt_mm, "csak_t")
            csbk_t = ptile([P, T], dt_mm, "csbk_t")
            bq_t = ptile([P, NPAIR], F32, "bq_t")
            bk_t = ptile([P, NPAIR], F32, "bk_t")
            bv_t = ptile([P, NPAIR], F32, "bv_t")
            ones_bf = ptile([P, HD], dt_mm, "ones_bf")
            attn = [ptile([P, TQ], dt_mm, f"attn{pr}") for pr in range(NPAIR)]
            wo_c = [ptile([P, D], dt_mm, f"wo{ch}") for ch in range(NPAIR)]
            bob_t = ptile([P, D], F32, "bob_t")
            nc.gpsimd.memset(ones_bf[:], 1.0)
            # small tiles + cos/sin go on the otherwise-idle gpsimd swdge
            # queue so the hwdge queues stay clear for pair-0's weights
            nc.gpsimd.dma_start(bq_t[:], bq[:])
            nc.gpsimd.dma_start(bk_t[:], bk[:])
            nc.gpsimd.dma_start(bv_t[:], bv[:])
            nc.gpsimd.dma_start(csak_t[:], csak[:])
            nc.gpsimd.dma_start(csbk_t[:], csbk[:])
            nc.gpsimd.dma_start(csaq_t[:], csaq[:])
            nc.gpsimd.dma_start(csbq_t[:], csbq[:])

            def emit_x_dmas():
                nc.sync.dma_start(xall[:, 0], xd[:, 0])
                nc.scalar.dma_start(xall[:, 1], xd[:, 1])
                nc.sync.dma_start(xall[:, 2], xd[:, 2])
                nc.scalar.dma_start(xall[:, 3], xd[:, 3])

            def emit_wo_dmas():
                for ch in range(NPAIR):
                    eng = nc.sync if ch % 2 == 0 else nc.scalar
                    eng.dma_start(wo_c[ch][:], wot[ch * P : (ch + 1) * P, :])
                nc.gpsimd.dma_start(bob_t[:], bob[:])

            # ---- pools for the head-pair loop ----
            big = 2 if dt_mm != F32 else 1
            wp = ctx.enter_context(tc.tile_pool(name="wp", bufs=2))
            rawp = ctx.enter_context(tc.tile_pool(name="rawp", bufs=2))
            ropep = ctx.enter_context(tc.tile_pool(name="ropep", bufs=2))
            vtp = ctx.enter_context(tc.tile_pool(name="vtp", bufs=1))
            qkp = ctx.enter_context(tc.tile_pool(name="qkp", bufs=big))
            vnp = ctx.enter_context(tc.tile_pool(name="vnp", bufs=big))
            exp_p = ctx.enter_context(tc.tile_pool(name="exp_p", bufs=2))
            sumsp = ctx.enter_context(tc.tile_pool(name="sumsp", bufs=2))
            recp = ctx.enter_context(tc.tile_pool(name="recp", bufs=2))
            ps_proj = ctx.enter_context(
                tc.tile_pool(name="ps_proj", bufs=2, space="PSUM")
            )
            ps_sc = ctx.enter_context(
                tc.tile_pool(name="ps_sc", bufs=1, space="PSUM")
            )
            ps_po = ctx.enter_context(
                tc.tile_pool(name="ps_po", bufs=1, space="PSUM")
            )

            def rope_chunk(raw, cs_a, cs_b, out_tile, cols):
                """out[:, cols] = raw[:, cols]*csa + swap32(raw*csb) via 4
                partition-offset muls: both inputs read at the same base
                (walrus requires equal SBUF input bases), only the output
                partition is offset.  Chunked so the DVE work spreads across
                the attention window instead of clustering at pair end."""
                m1 = ropep.tile([P, NQ], dt_mm, tag="m1", name="m1")
                m2 = ropep.tile([P, NQ], dt_mm, tag="m2", name="m2")
                nc.vector.tensor_mul(m1[:], raw[:, cols], cs_a[:, cols])
                for blk in range(4):
                    o0 = blk * 32
                    i0 = o0 + 32 if blk % 2 == 0 else o0 - 32
                    nc.vector.tensor_mul(
                        m2[o0 : o0 + 32, :],
                        raw[i0 : i0 + 32, cols],
                        cs_b[i0 : i0 + 32, cols],
                    )
                nc.vector.tensor_add(out_tile[:, cols], m1[:], m2[:])

            def stage_units(pr, eager_wdma=False):
                """Emission units for pair pr's projections + RoPE + V
                transpose, pumped into the previous pair's attention.  V^T
                tiles are produced by the DMA XBAR transpose (no PE/copy
                involvement)."""
                st = {}
                units = []

                def u_wdma():
                    st["wq"] = wp.tile([P, NPAIR, P], dt_mm, tag="wq", name="wq_c")
                    st["wk"] = wp.tile([P, NPAIR, P], dt_mm, tag="wk", name="wk_c")
                    st["wv"] = wp.tile([P, NPAIR, P], dt_mm, tag="wv", name="wv_c")
                    nc.sync.dma_start(st["wk"][:], wkt[:, pr])
                    nc.scalar.dma_start(st["wv"][:], wvt[:, pr])
                    nc.sync.dma_start(st["wq"][:], wqt[:, pr])
                    st["qraw"] = rawp.tile([P, TQ], dt_mm, tag="qraw", name="q_raw")
                    st["kraw"] = rawp.tile([P, T], dt_mm, tag="kraw", name="k_raw")
                    st["vt"] = vtp.tile([P, T], dt_mm, tag="vt", name="v_t")
                    st["qt"] = qkp.tile([P, TQ], dt_mm, tag="qt", name="qt")
                    st["kt"] = qkp.tile([P, T], dt_mm, tag="kt", name="kt")
                    for hh in range(2):
                        vn_h = vnp.tile(
                            [P, T // P, P], dt_mm, tag=f"vn{hh}", name="vn_h"
                        )
                        # col 64 = ones (softmax denominator); 65.. = zero
                        nc.gpsimd.memset(vn_h[:, :, HD : HD + 1], 1.0)
                        nc.gpsimd.memset(vn_h[:, :, HD + 1 :], 0.0)
                        st[f"vn{hh}"] = vn_h

                if eager_wdma:
                    u_wdma()
                else:
                    units.append(u_wdma)

                def u_mm(w_key, nb, f, start, stop):
                    def go():
                        if start:
                            st["ps"] = ps_proj.tile([P, NQ], F32, tag="ps", name="ps")
                        nc.tensor.matmul(
                            st["ps"][:],
                            st[w_key][:, f, :],
                            xall[:, nb, f, :],
                            start=start,
                            stop=stop,
                        )

                    return go

                def u_evict(b_t, dst_key, dslice):
                    def go():
                        nc.scalar.activation(
                            st[dst_key][:, dslice],
                            st["ps"][:],
                            mybir.ActivationFunctionType.Identity,
                            bias=b_t[:, pr : pr + 1],
                        )

                    return go

                def u_ropek(nb):
                    def go():
                        rope_chunk(
                            st["kraw"], csak_t, csbk_t, st["kt"],
                            slice(nb * NQ, (nb + 1) * NQ),
                        )

                    return go

                def u_ropeq(nb):
                    def go():
                        rope_chunk(
                            st["qraw"], csaq_t, csbq_t, st["qt"],
                            slice(nb * NQ, (nb + 1) * NQ),
                        )

                    return go

                def u_vtr(hh, nb):
                    def go():
                        h0 = hh * HD
                        eng = nc.sync if hh == 0 else nc.scalar
                        eng.dma_start_transpose(
                            st[f"vn{hh}"][:, 4 * nb : 4 * nb + 4, :HD],
                            st["vt"][h0 : h0 + HD, nb * NQ : (nb + 1) * NQ],
                        )

                    return go

                for nb in range(NB):
                    cols = slice(nb * NQ, (nb + 1) * NQ)
                    for f in range(NPAIR):
                        units.append(u_mm("wk", nb, f, f == 0, f == NPAIR - 1))
                    units.append(u_evict(bk_t, "kraw", cols))
                    units.append(u_ropek(nb))
                    for f in range(NPAIR):
                        units.append(u_mm("wv", nb, f, f == 0, f == NPAIR - 1))
                    units.append(u_evict(bv_t, "vt", cols))
                    units.append(u_vtr(0, nb))
                    units.append(u_vtr(1, nb))
                # Q projection reuses x slabs 0-1 (own query half, host-permuted)
                for nb in range(TQ // NQ):
                    for f in range(NPAIR):
                        units.append(u_mm("wq", nb, f, f == 0, f == NPAIR - 1))
                    units.append(u_evict(bq_t, "qraw", slice(nb * NQ, (nb + 1) * NQ)))
                    units.append(u_ropeq(nb))
                return st, units

            def pump(units, n):
                for _ in range(n):
                    if units:
                        units.pop(0)()

            def attention(pr, st, next_units, pump_rate):
                """Attention for pair pr, both heads interleaved: their
                contraction-64 scores matmuls are adjacent and land in
                disjoint PE row-halves.  PSUM is evicted unnormalized; one
                batched fast-reciprocal + GpSimd partition broadcasts + 4
                in-place muls normalize at pair end."""
                # denominator rows live at partitions 0/32/64/96 (SBUF
                # engine accesses must start at 32-aligned partitions)
                sums = sumsp.tile([P, NQ], F32, tag="sums", name="sums")
                for qb in range(TQ // NQ):
                    qs = slice(qb * NQ, (qb + 1) * NQ)
                    po = [
                        ps_po.tile([P, NQ], F32, tag=f"po{hh}", name="po")
                        for hh in range(2)
                    ]
                    pending_pv = None
                    for ci in range(T // P // 2):
                        ps2 = [
                            ps_sc.tile([P, 2 * NQ], F32, tag=f"sc{hh}", name="ps2")
                            for hh in range(2)
                        ]
                        for k in range(2):
                            ch = 2 * ci + k
                            for hh in range(2):
                                h0 = hh * HD
                                nc.tensor.matmul(
                                    ps2[hh][:, k * NQ : (k + 1) * NQ],
                                    st["kt"][h0 : h0 + HD, ch * P : (ch + 1) * P],
                                    st["qt"][h0 : h0 + HD, qs],
                                    start=True,
                                    stop=True,
                                )
                        pexp = [
                            exp_p.tile([P, 2 * NQ], dt_mm, tag=f"ex{hh}", name="pexp")
                            for hh in range(2)
                        ]
                        for hh in range(2):
                            nc.scalar.activation(
                                pexp[hh][:],
                                ps2[hh][:],
                                mybir.ActivationFunctionType.Exp,
                                scale=float(SCALE),
                            )
                        pump(next_units, pump_rate)
                        # PV runs one iteration behind so exp has a full
                        # iteration of latency to hide
                        if pending_pv is not None:
                            pending_pv()

                        def make_pv(pexp=pexp, ci=ci):
                            def go():
                                for k in range(2):
                                    ch = 2 * ci + k
                                    for hh in range(2):
                                        nc.tensor.matmul(
                                            po[hh][:],
                                            st[f"vn{hh}"][:, ch, :],
                                            pexp[hh][:, k * NQ : (k + 1) * NQ],
                                            start=(ch == 0),
                                            stop=(ch == T // P - 1),
                                        )

                            return go

                        pending_pv = make_pv()
                    pending_pv()
                    # evict unnormalized attn rows + denominator row
                    for hh in range(2):
                        nc.vector.tensor_copy(
                            attn[pr][hh * HD : (hh + 1) * HD, qs], po[hh][:HD, :]
                        )
                        seg0 = (qb * 2 + hh) * 32
                        nc.vector.tensor_copy(
                            sums[seg0 : seg0 + 1, :],
                            po[hh][HD : HD + 1, :],
                        )
                # batched fast reciprocal over the pair's four denominator
                # rows (lanes between them hold garbage and are never read),
                # then a K=1 bf16 matmul broadcast per segment (ones slice at
                # the matching base partition keeps walrus happy)
                rec = recp.tile([P, NQ], F32, tag="rec", name="rec")
                rec_bf = recp.tile([P, NQ], dt_mm, tag="rec_bf", name="rec_bf")
                # reciprocal cost scales with free size only: the [128, 512]
                # layout is 4x cheaper than the [1, 2048] single-lane form
                # (only rows 0/32/64/96 hold real denominators)
                nc.vector.reciprocal(rec[:], sums[:])
                nc.vector.tensor_copy(rec_bf[:], rec[:])
                for qb in range(TQ // NQ):
                    qs = slice(qb * NQ, (qb + 1) * NQ)
                    recb = recp.tile([P, NQ], F32, tag="recb", name="recb")
                    for hh in range(2):
                        seg0 = (qb * 2 + hh) * 32
                        pb = ps_sc.tile([P, NQ], F32, tag=f"sc{hh}", name="pb")
                        nc.tensor.matmul(
                            pb[:HD, :],
                            ones_bf[seg0 : seg0 + 1, :],
                            rec_bf[seg0 : seg0 + 1, :],
                            start=True,
                            stop=True,
                            tile_position=(seg0, 0),
                        )
                        nc.vector.tensor_copy(
                            recb[hh * HD : (hh + 1) * HD, :], pb[:HD, :]
                        )
                        nc.vector.tensor_mul(
                            attn[pr][hh * HD : (hh + 1) * HD, qs],
                            attn[pr][hh * HD : (hh + 1) * HD, qs],
                            recb[hh * HD : (hh + 1) * HD, :],
                        )

            st, units = stage_units(0, eager_wdma=True)
            emit_x_dmas()
            pump(units, len(units))
            # wo/bob prefetch queues behind all of pair-0's critical DMAs
            emit_wo_dmas()
            for pr in range(NPAIR):
                if pr + 1 < NPAIR:
                    nxt_st, nxt_units = stage_units(pr + 1)
                else:
                    nxt_st, nxt_units = None, []
                n_pumps = (TQ // NQ) * (T // P // 2)  # qb x ci pump points
                pump_rate = (len(nxt_units) + n_pumps - 1) // n_pumps if nxt_units else 0
                attention(pr, st, nxt_units, pump_rate)
                pump(nxt_units, len(nxt_units))
                st = nxt_st

        # ---- output projection (separate pool scope) ----
        with contextlib.ExitStack() as ctx:
            outp = ctx.enter_context(tc.tile_pool(name="outp", bufs=3))
            ps_o = ctx.enter_context(
                tc.tile_pool(name="ps_o", bufs=8, space="PSUM")
            )
            for tb in range(TQ // P):
                ts = slice(tb * P, (tb + 1) * P)
                pout = [
                    ps_o.tile([P, NQ], F32, tag="pout", name="pout")
                    for _ in range(2)
                ]
                for ch in range(NPAIR):
                    for nh in range(2):
                        nc.tensor.matmul(
                            pout[nh][:],
                            attn[ch][:, ts],
                            wo_c[ch][:, nh * NQ : (nh + 1) * NQ],
                            start=(ch == 0),
                            stop=(ch == NPAIR - 1),
                        )
                osb = outp.tile([P, D], F32, tag="osb", name="osb")
                for nh in range(2):
                    nc.vector.tensor_add(
                        osb[:, nh * NQ : (nh + 1) * NQ],
                        pout[nh][:],
                        bob_t[:, nh * NQ : (nh + 1) * NQ],
                    )
                nc.sync.dma_start(out[ts, :], osb[:])

        persist_cm.__exit__(None, None, None)

    _split_sync_waits(nc)
    return nc


# ---------------- host-side input prep ----------------
def _np_dt(dt_mm):
    return ml_dtypes.bfloat16 if dt_mm == mybir.dt.bfloat16 else np.float32


def _cs_tiles(frac_b):
    """csa/csb [128, T] f32 RoPE tiles for one batch (frac_b: [T] f32).
    csb rows follow raw's layout ([sin; -sin] x2): the device reads
    raw*csb at the source base and writes to the swapped partition."""
    i = np.arange(HALF, dtype=np.float64)
    freq = (ROPE_BASE ** (2.0 * i / HD)).astype(np.float32)  # [32]
    pos = frac_b.astype(np.float32) * np.float32(ROPE_SCALE)
    ang = pos[None, :] / freq[:, None]  # [32, T] f32
    a64 = ang.astype(np.float64)
    cos = np.cos(a64).astype(np.float32)
    sin = np.sin(a64).astype(np.float32)
    csa = np.tile(cos, (4, 1))  # [128, T]
    csb = np.tile(np.concatenate([sin, -sin], axis=0), (2, 1))  # [128, T]
    return np.ascontiguousarray(csa), np.ascontiguousarray(csb)


def _pre_tile_w(Wt, npdt):
    """W.T [D, D] -> [p, pair, f, d] so each per-pair DMA line is 2KB."""
    return np.ascontiguousarray(
        Wt.reshape(NPAIR, P, NPAIR, P).transpose(1, 2, 0, 3)
    ).astype(npdt)


def make_in_maps(x, frac, Wq, bq, Wk, bk, Wv, bv, Wo, bo, dt_mm=DT_MM):
    npdt = _np_dt(dt_mm)
    wqt = _pre_tile_w(np.ascontiguousarray(Wq.T), npdt)
    wkt = _pre_tile_w(np.ascontiguousarray(Wk.T), npdt)
    wvt = _pre_tile_w(np.ascontiguousarray(Wv.T), npdt)
    wot = np.ascontiguousarray(Wo.T).astype(npdt)
    bq_t = np.ascontiguousarray(bq.reshape(NPAIR, P).T).astype(np.float32)
    bk_t = np.ascontiguousarray(bk.reshape(NPAIR, P).T).astype(np.float32)
    bv_t = np.ascontiguousarray(bv.reshape(NPAIR, P).T).astype(np.float32)
    bob = np.ascontiguousarray(np.tile(bo[None, :], (P, 1))).astype(np.float32)
    in_maps = []
    for c in range(N_CORES):
        b, tqh = c // 2, c % 2
        # chunk order: own query half first (slabs 0-1), so Q-proj reuses them
        perm = [0, 1, 2, 3] if tqh == 0 else [2, 3, 0, 1]
        xt = np.ascontiguousarray(x[b].T)  # [D, T] f32
        xd = (
            xt.reshape(NPAIR, P, NB, NQ)[:, :, perm, :]
            .transpose(1, 2, 0, 3)
            .astype(npdt)
        )  # [p, nb, f, t]
        csa, csb_sw = _cs_tiles(frac[b])
        csa_k = csa.reshape(P, NB, NQ)[:, perm, :].reshape(P, T)
        csb_k = csb_sw.reshape(P, NB, NQ)[:, perm, :].reshape(P, T)
        in_maps.append(
            {
                "xd": np.ascontiguousarray(xd),
                "wqt": wqt,
                "wkt": wkt,
                "wvt": wvt,
                "wot": wot,
                "bq": bq_t,
                "bk": bk_t,
                "bv": bv_t,
                "bob": bob,
                "csaq": np.ascontiguousarray(
                    csa[:, tqh * TQ : (tqh + 1) * TQ]
                ).astype(npdt),
                "csbq": np.ascontiguousarray(
                    csb_sw[:, tqh * TQ : (tqh + 1) * TQ]
                ).astype(npdt),
                "csak": np.ascontiguousarray(csa_k).astype(npdt),
                "csbk": np.ascontiguousarray(csb_k).astype(npdt),
            }
        )
    return in_maps


_NC_CACHE = {}


def _get_nc(dt_mm=DT_MM):
    key = str(dt_mm)
    if key not in _NC_CACHE:
        _NC_CACHE[key] = build_nc(dt_mm)
    return _NC_CACHE[key]


def kernel(x, frac, Wq, bq, Wk, bk, Wv, bv, Wo, bo):
    install_shims()
    from concourse.bass_utils import run_bass_kernel_spmd

    x = np.asarray(x, dtype=np.float32)
    frac = np.asarray(frac, dtype=np.float32)
    args = [np.asarray(a, dtype=np.float32) for a in (Wq, bq, Wk, bk, Wv, bv, Wo, bo)]
    in_maps = make_in_maps(x, frac, *args, dt_mm=DT_MM)
    nc = _get_nc(DT_MM)
    res = run_bass_kernel_spmd(nc, in_maps, list(range(N_CORES)))
    out = np.empty((B, T, D), dtype=np.float32)
    for c in range(N_CORES):
        b, tqh = c // 2, c % 2
        out[b, tqh * TQ : (tqh + 1) * TQ, :] = res.results[c]["out"]
    return out
